# revision 2
# baseline (speedup 1.0000x reference)
"""MoE (top-2 of 8 experts) Trainium2 kernel, 8-core data-parallel over tokens.

Problem shapes (hardcoded): x [4, 2048, 512] f32, Wg [512, 8], W1 [8, 512, 1024],
b1 [8, 1024], W2 [8, 1024, 512], b2 [8, 512].  T = 8192 tokens, top-2 routing.

Strategy: shard tokens across the 8 cores (1024/core); replicate router and
expert weights (weights cast to bf16 host-side).  Fully on device, per core:
  1. xT via PE transpose; fp32 router -> softmax -> top-2 (DVE max8).
  2. Per 128-token tile: within-tile token rank per expert via a
     triangular-ones matmul prefix-sum; slot = e*CAP + tile*CAPT + rank
     (per-tile local capacity CAPT=48, so tiles dispatch independently);
     bf16 x rows scattered to the slot via indirect DMA.  No zero-init of the
     staging buffer: padded slots compute garbage that is never gathered.
  3. Per expert (staging fused into the loop so DMA/PE pipeline across
     experts): load its <=CAP staged rows, PE-transpose, bf16 GEMM1 -> fused
     gelu_tanh(+b1) -> bf16 GEMM2 (+b2), write y rows slot-ordered in bf16.
  4. Final combine per token tile: indirect-gather the token's two y rows by
     the saved slot ids, scale by the (f32) gates, add, write out.
Weights prefetch 3 experts deep so weight DMA overlaps the router phase.
"""

from contextlib import ExitStack

import numpy as np
import ml_dtypes

import concourse.bass as bass
import concourse.tile as tile
from concourse import bacc, mybir
from concourse.bass import IndirectOffsetOnAxis
from concourse.bass_utils import run_bass_kernel_spmd
from concourse.masks import make_identity

P = 128
N_CORES = 8
B, S, D, H, O, E = 4, 2048, 512, 1024, 512, 8
T = B * S                    # 8192
TC = T // N_CORES            # 1024 tokens per core
DC = D // P                  # 4 D-chunks
HC = H // P                  # 8 H-chunks
NT = TC // P                 # 8 token tiles of 128
CAP = 384                    # per-expert token capacity (3 tiles of 128)
NS = CAP // P                # 3 slot tiles per expert
CAPT = CAP // NT             # 48: per-(tile, expert) local capacity

MM_DT = mybir.dt.bfloat16
NP_MM_DT = ml_dtypes.bfloat16
F32 = mybir.dt.float32
I32 = mybir.dt.int32
AF = mybir.ActivationFunctionType
ALU = mybir.AluOpType


def build_nc(has_b1: bool, has_b2: bool) -> bass.Bass:
    nc = bacc.Bacc()
    x_d = nc.declare_dram_parameter("x", [TC, D], F32, isOutput=False)
    wg_d = nc.declare_dram_parameter("wg", [D, E], F32, isOutput=False)
    w1_d = nc.declare_dram_parameter("w1", [E, D, H], MM_DT, isOutput=False)
    w2_d = nc.declare_dram_parameter("w2", [E, H, O], MM_DT, isOutput=False)
    if has_b1:
        b1_d = nc.declare_dram_parameter("b1", [E, H], F32, isOutput=False)
    if has_b2:
        b2_d = nc.declare_dram_parameter("b2", [E, O], F32, isOutput=False)
    out_d = nc.declare_dram_parameter("out", [TC, O], F32, isOutput=True)

    xg_d = nc.dram_tensor("xg", [E * CAP, D], MM_DT)
    y_d = nc.dram_tensor("yd", [E * CAP, O], MM_DT)

    with ExitStack() as ctx:
        tc = ctx.enter_context(tile.TileContext(nc))
        singles = ctx.enter_context(tc.tile_pool(name="singles", bufs=1))
        xload = ctx.enter_context(tc.tile_pool(name="xload", bufs=4))
        wpool = ctx.enter_context(tc.tile_pool(name="wpool", bufs=6))
        xtg = ctx.enter_context(tc.tile_pool(name="xtg", bufs=3))
        hpool = ctx.enter_context(tc.tile_pool(name="hpool", bufs=2))
        tmp = ctx.enter_context(tc.tile_pool(name="tmp", bufs=6))
        ypool = ctx.enter_context(tc.tile_pool(name="ypool", bufs=4))
        psum_t = ctx.enter_context(tc.tile_pool(name="psum_t", bufs=2, space="PSUM"))
        psum_r = ctx.enter_context(tc.tile_pool(name="psum_r", bufs=1, space="PSUM"))
        psum_rk = ctx.enter_context(tc.tile_pool(name="psum_rk", bufs=1, space="PSUM"))
        psum_h = ctx.enter_context(tc.tile_pool(name="psum_h", bufs=2, space="PSUM"))
        psum_y = ctx.enter_context(tc.tile_pool(name="psum_y", bufs=2, space="PSUM"))

        ident = singles.tile([P, P], F32)
        make_identity(nc, ident)
        ident16 = singles.tile([P, P], MM_DT)
        nc.vector.tensor_copy(ident16, ident)

        # inclusive lower-triangular ones: tril[q, p] = 1.0 iff q <= p
        tril = singles.tile([P, P], F32)
        nc.gpsimd.memset(tril, 0.0)
        nc.gpsimd.affine_select(
            out=tril, in_=tril, compare_op=ALU.is_gt, fill=1.0,
            base=0, pattern=[[-1, P]], channel_multiplier=1,
        )

        wg_sb = singles.tile([P, DC, E], F32)
        nc.sync.dma_start(wg_sb, wg_d[:].rearrange("(c p) e -> p c e", p=P))
        if has_b1:
            b1_sb = singles.tile([P, HC, E], F32)
            with nc.allow_non_contiguous_dma(reason="tiny one-time b1 load"):
                nc.sync.dma_start(b1_sb, b1_d[:].rearrange("e (c p) -> p c e", p=P))
        if has_b2:
            b2_sb = singles.tile([P, E, O], F32)
            b2_ap = b2_d[:]
            b2_bcast = bass.AP(
                tensor=b2_ap.tensor, offset=b2_ap.offset, ap=[[0, P], *b2_ap.ap]
            )
            nc.sync.dma_start(b2_sb, b2_bcast)

        # iotas: per-(tile, expert) slot bases
        iota_base_i = singles.tile([P, NT, E], I32)
        nc.gpsimd.iota(
            iota_base_i, pattern=[[CAPT, NT], [CAP, E]], base=0, channel_multiplier=0
        )
        iota_base = singles.tile([P, NT, E], F32)
        nc.vector.tensor_copy(iota_base, iota_base_i)

        xT32 = singles.tile([P, DC, TC], F32)
        x16_all = singles.tile([P, NT, D], MM_DT)
        slotk_all = singles.tile([P, NT, 2], I32)
        gates_all = singles.tile([P, NT, 2], F32)

        # ---- transpose x into xT (fp32, for router) + bf16 copy in SBUF ----
        for tt in range(NT):
            xr = xload.tile([P, D], F32, tag="xr")
            nc.sync.dma_start(xr, x_d[:][tt * P:(tt + 1) * P, :])
            nc.vector.tensor_copy(x16_all[:, tt, :], xr)
            for dc in range(DC):
                pt = psum_t.tile([P, P], F32, tag="pt")
                nc.tensor.transpose(pt, xr[:, dc * P:(dc + 1) * P], ident)
                nc.vector.tensor_copy(xT32[:, dc, tt * P:(tt + 1) * P], pt)

        # ---- per tile: router, top-2, local rank, slots, dispatch scatters ----
        for tt in range(NT):
            pr = psum_r.tile([P, E], F32, tag="pr")
            for dc in range(DC):
                nc.tensor.matmul(
                    pr, lhsT=xT32[:, dc, tt * P:(tt + 1) * P], rhs=wg_sb[:, dc, :],
                    start=(dc == 0), stop=(dc == DC - 1),
                )
            # top-2 selection runs on unnormalized exp(logits); the softmax
            # denominator only scales the two gate values at pair-copy time.
            ex = tmp.tile([P, E], F32, tag="ex")
            s = tmp.tile([P, 1], F32, tag="s")
            nc.scalar.activation(out=ex, in_=pr, func=AF.Exp, accum_out=s)
            rec = tmp.tile([P, 1], F32, tag="rec")
            nc.vector.reciprocal(rec, s)
            top8 = tmp.tile([P, 8], F32, tag="top8")
            nc.vector.max(out=top8, in_=ex)
            mask = tmp.tile([P, E], F32, tag="mask")
            nc.vector.tensor_scalar(
                out=mask, in0=ex, scalar1=top8[:, 1:2], scalar2=None, op0=ALU.is_ge
            )
            # within-tile inclusive rank via triangular-ones matmul
            prk = psum_rk.tile([P, E], F32, tag="prk")
            nc.tensor.matmul(prk, lhsT=tril, rhs=mask, start=True, stop=True)

            slots = tmp.tile([P, E], F32, tag="slots")
            nc.vector.tensor_sub(slots, prk, mask)  # exclusive rank
            nc.vector.tensor_add(slots, slots, iota_base[:, tt, :])
            oh1 = tmp.tile([P, E], F32, tag="oh1")
            nc.vector.tensor_scalar(
                out=oh1, in0=ex, scalar1=top8[:, 0:1], scalar2=None, op0=ALU.is_equal
            )
            sel = tmp.tile([P, E], F32, tag="sel")
            slotk_f = tmp.tile([P, 2], F32, tag="slotk_f")
            nc.vector.tensor_mul(sel, oh1, slots)
            nc.vector.reduce_sum(slotk_f[:, 0:1], sel, axis=mybir.AxisListType.X)
            nc.vector.tensor_sub(sel, mask, oh1)  # top-2 one-hot
            nc.vector.tensor_mul(sel, sel, slots)
            nc.vector.reduce_sum(slotk_f[:, 1:2], sel, axis=mybir.AxisListType.X)
            slotk_i = slotk_all[:, tt, :]
            nc.vector.tensor_copy(slotk_i, slotk_f)

            nc.vector.tensor_scalar_mul(gates_all[:, tt, :], top8[:, 0:2], rec)
            for k in range(2):
                nc.gpsimd.indirect_dma_start(
                    out=xg_d[:],
                    out_offset=IndirectOffsetOnAxis(
                        ap=slotk_i[:, k:k + 1], axis=0
                    ),
                    in_=x16_all[:, tt, :],
                    in_offset=None,
                    bounds_check=E * CAP - 1,
                    oob_is_err=False,
                )

        # ---- per expert: stage rows + transpose, MLP, y rows slot-ordered ----
        for e in range(E):
            w1_sb = wpool.tile([P, DC, H], MM_DT, tag="w1")
            nc.sync.dma_start(w1_sb, w1_d[:][e].rearrange("(c p) h -> p c h", p=P))
            w2_sb = wpool.tile([P, HC, O], MM_DT, tag="w2")
            nc.sync.dma_start(w2_sb, w2_d[:][e].rearrange("(c p) o -> p c o", p=P))

            xTg = xtg.tile([P, DC, CAP], MM_DT, tag="xTg")
            for sl in range(NS):
                xg = xload.tile([P, D], MM_DT, tag="xg")
                nc.sync.dma_start(
                    xg, xg_d[:][e * CAP + sl * P:e * CAP + (sl + 1) * P, :]
                )
                for dc in range(DC):
                    pt16 = psum_t.tile([P, P], MM_DT, tag="pt")
                    nc.tensor.transpose(pt16, xg[:, dc * P:(dc + 1) * P], ident16)
                    nc.vector.tensor_copy(
                        xTg[:, dc, sl * P:(sl + 1) * P], pt16
                    )

            h_sb = hpool.tile([P, HC, CAP], MM_DT, tag="h")
            for hc in range(HC):
                ph = psum_h.tile([P, CAP], F32)
                for dc in range(DC):
                    nc.tensor.matmul(
                        ph, lhsT=w1_sb[:, dc, hc * P:(hc + 1) * P],
                        rhs=xTg[:, dc, :],
                        start=(dc == 0), stop=(dc == DC - 1),
                    )
                bias_ap = b1_sb[:, hc, e:e + 1] if has_b1 else 0.0
                nc.scalar.activation(
                    out=h_sb[:, hc, :], in_=ph, func=AF.Gelu_apprx_tanh, bias=bias_ap
                )

            for sl in range(NS):
                py = psum_y.tile([P, O], F32)
                for hc in range(HC):
                    nc.tensor.matmul(
                        py, lhsT=h_sb[:, hc, sl * P:(sl + 1) * P], rhs=w2_sb[:, hc, :],
                        start=(hc == 0), stop=(hc == HC - 1),
                    )
                yg = ypool.tile([P, O], MM_DT, tag="yg")
                if has_b2:
                    nc.vector.tensor_add(yg, py, b2_sb[:, e, :])
                else:
                    nc.vector.tensor_copy(yg, py)
                nc.sync.dma_start(
                    y_d[:][e * CAP + sl * P:e * CAP + (sl + 1) * P, :], yg
                )

        # ---- final combine: per token, gather its two y rows, gate, add ----
        for tt in range(NT):
            y1 = xload.tile([P, O], MM_DT, tag="y1")
            nc.gpsimd.indirect_dma_start(
                out=y1,
                out_offset=None,
                in_=y_d[:],
                in_offset=IndirectOffsetOnAxis(
                    ap=slotk_all[:, tt, 0:1], axis=0
                ),
                bounds_check=E * CAP - 1,
                oob_is_err=False,
            )
            y2 = xload.tile([P, O], MM_DT, tag="y2")
            nc.gpsimd.indirect_dma_start(
                out=y2,
                out_offset=None,
                in_=y_d[:],
                in_offset=IndirectOffsetOnAxis(
                    ap=slotk_all[:, tt, 1:2], axis=0
                ),
                bounds_check=E * CAP - 1,
                oob_is_err=False,
            )
            yo1 = ypool.tile([P, O], F32, tag="yo1")
            yo2 = ypool.tile([P, O], F32, tag="yo2")
            nc.vector.tensor_scalar_mul(yo1, y1, gates_all[:, tt, 0:1])
            nc.vector.tensor_scalar_mul(yo2, y2, gates_all[:, tt, 1:2])
            nc.vector.tensor_add(yo1, yo1, yo2)
            nc.sync.dma_start(out_d[:][tt * P:(tt + 1) * P, :], yo1)

    nc.finalize()
    return nc


_NC_CACHE: dict = {}


def _get_nc(has_b1: bool, has_b2: bool) -> bass.Bass:
    key = (has_b1, has_b2)
    if key not in _NC_CACHE:
        _NC_CACHE[key] = build_nc(has_b1, has_b2)
    return _NC_CACHE[key]


def kernel(x, Wg, W1, b1, W2, b2, _trace=False, _tmpdir=None):
    x = np.ascontiguousarray(np.asarray(x, dtype=np.float32))
    Wg = np.ascontiguousarray(np.asarray(Wg, dtype=np.float32))
    W1 = np.asarray(W1, dtype=np.float32)
    b1 = np.asarray(b1, dtype=np.float32)
    W2 = np.asarray(W2, dtype=np.float32)
    b2 = np.asarray(b2, dtype=np.float32)

    has_b1 = bool(np.any(b1))
    has_b2 = bool(np.any(b2))
    nc = _get_nc(has_b1, has_b2)

    xm = x.reshape(T, D)
    w1_bf = np.ascontiguousarray(W1.astype(NP_MM_DT))
    w2_bf = np.ascontiguousarray(W2.astype(NP_MM_DT))

    base = {"wg": Wg, "w1": w1_bf, "w2": w2_bf}
    if has_b1:
        base["b1"] = np.ascontiguousarray(b1)
    if has_b2:
        base["b2"] = np.ascontiguousarray(b2)

    in_maps = [
        {**base, "x": np.ascontiguousarray(xm[c * TC:(c + 1) * TC])}
        for c in range(N_CORES)
    ]
    res = run_bass_kernel_spmd(
        nc, in_maps, core_ids=list(range(N_CORES)), trace=_trace, tmpdir=_tmpdir
    )
    out = np.concatenate([res.results[c]["out"] for c in range(N_CORES)], axis=0)
    if _trace:
        kernel._last_result = res
    return out.reshape(B, S, O).astype(np.float32)


# revision 18
# speedup vs baseline: 1.2237x; 1.2237x over previous
"""MoE (top-2 of 8 experts) Trainium2 kernel, 8-core data-parallel over tokens.

Problem shapes (hardcoded): x [4, 2048, 512] f32, Wg [512, 8], W1 [8, 512, 1024],
b1 [8, 1024], W2 [8, 1024, 512], b2 [8, 512].  T = 8192 tokens, top-2 routing.

Strategy: shard tokens across the 8 cores (1024/core); replicate router and
expert weights (weights cast to bf16 host-side).  Fully on device, per core:
  1. Per 128-token tile: load x, PE-transpose (f32 for the router), router
     matmuls batched into one PSUM bank region per tile so the PE never
     stalls on the DVE softmax chain.
  2. Per tile: softmax/top-2 on DVE; within-tile rank via triangular-ones
     matmul; slot = e*CAP + tile*CAPT + rank (CAPT=48 per-tile capacity);
     two indirect scatters per tile send the top-k x rows (bf16) to
     xg[slot].  No zero-init: padded slots compute garbage that is never
     gathered at the combine.
  3. Staging pass: per (expert, slot-tile): load staged rows, PE-transpose
     into a resident xTg buffer (kept out of the GEMM stream).
  4. Per expert: bf16 GEMM1 -> fused gelu_tanh(+b1) -> bf16 GEMM2 (+b2);
     y rows written slot-ordered in bf16 (regular DMA).
  5. Combine per token tile: indirect-gather the token's two y rows by the
     saved slot ids, scale by f32 gates, add, store.
Weights prefetch ~2 experts deep (bufs=4 pools) so weight DMA paces out
across the dispatch phase instead of flooding it.
"""

from contextlib import ExitStack

import numpy as np
import ml_dtypes

import concourse.bass as bass
import concourse.tile as tile
from concourse import bacc, mybir
from concourse.bass import IndirectOffsetOnAxis
from concourse.bass_utils import run_bass_kernel_spmd
from concourse.masks import make_identity

P = 128
N_CORES = 8
B, S, D, H, O, E = 4, 2048, 512, 1024, 512, 8
T = B * S                    # 8192
TC = T // N_CORES            # 1024 tokens per core
DC = D // P                  # 4 D-chunks
HC = H // P                  # 8 H-chunks
NT = TC // P                 # 8 token tiles of 128
CAP = 384                    # per-expert token capacity (3 tiles of 128)
NS = CAP // P                # 3 slot tiles per expert
CAPT = CAP // NT             # 48: per-(tile, expert) local capacity
MM_DT = mybir.dt.bfloat16
NP_MM_DT = ml_dtypes.bfloat16
F32 = mybir.dt.float32
I32 = mybir.dt.int32
AF = mybir.ActivationFunctionType
ALU = mybir.AluOpType
ACT_FN = AF.Gelu_apprx_tanh  # simtest.py swaps this for Tanh (sim support)


def _bcast2(ap: bass.AP, n: int) -> bass.AP:
    """Insert a stride-0 dim of size n after the partition dim."""
    return bass.AP(tensor=ap.tensor, offset=ap.offset, ap=[ap.ap[0], [0, n], *ap.ap[1:]])


def build_nc(has_b1: bool, has_b2: bool) -> bass.Bass:
    nc = bacc.Bacc()
    x_d = nc.declare_dram_parameter("x", [TC, D], F32, isOutput=False)
    wg_d = nc.declare_dram_parameter("wg", [D, E], F32, isOutput=False)
    w1_d = nc.declare_dram_parameter("w1", [E, D, H], MM_DT, isOutput=False)
    w2_d = nc.declare_dram_parameter("w2", [E, H, O], MM_DT, isOutput=False)
    if has_b1:
        b1_d = nc.declare_dram_parameter("b1", [E, H], F32, isOutput=False)
    if has_b2:
        b2_d = nc.declare_dram_parameter("b2", [E, O], F32, isOutput=False)
    out_d = nc.declare_dram_parameter("out", [TC, O], F32, isOutput=True)

    xg_d = nc.dram_tensor("xg", [E * CAP, D], MM_DT)
    y_d = nc.dram_tensor("yd", [E * CAP, O], MM_DT)

    with ExitStack() as ctx:
        tc = ctx.enter_context(tile.TileContext(nc))
        singles = ctx.enter_context(tc.tile_pool(name="singles", bufs=1))
        xload = ctx.enter_context(tc.tile_pool(name="xload", bufs=4))
        w1pool = ctx.enter_context(tc.tile_pool(name="w1pool", bufs=4))
        w2pool = ctx.enter_context(tc.tile_pool(name="w2pool", bufs=4))
        hpool = ctx.enter_context(tc.tile_pool(name="hpool", bufs=2))
        tmp = ctx.enter_context(tc.tile_pool(name="tmp", bufs=6))
        ypool = ctx.enter_context(tc.tile_pool(name="ypool", bufs=4))
        opool = ctx.enter_context(tc.tile_pool(name="opool", bufs=8))
        psum_t = ctx.enter_context(tc.tile_pool(name="psum_t", bufs=2, space="PSUM"))
        psum_r = ctx.enter_context(tc.tile_pool(name="psum_r", bufs=1, space="PSUM"))
        psum_h = ctx.enter_context(tc.tile_pool(name="psum_h", bufs=3, space="PSUM"))
        psum_y = ctx.enter_context(tc.tile_pool(name="psum_y", bufs=2, space="PSUM"))

        ident = singles.tile([P, P], F32)
        make_identity(nc, ident)
        ident16 = singles.tile([P, P], MM_DT)
        nc.vector.tensor_copy(ident16, ident)

        # inclusive lower-triangular ones: tril[q, p] = 1.0 iff q <= p
        tril = singles.tile([P, P], F32)
        nc.gpsimd.memset(tril, 0.0)
        nc.gpsimd.affine_select(
            out=tril, in_=tril, compare_op=ALU.is_gt, fill=1.0,
            base=0, pattern=[[-1, P]], channel_multiplier=1,
        )

        wg_sb = singles.tile([P, DC, E], F32)
        nc.sync.dma_start(wg_sb, wg_d[:].rearrange("(c p) e -> p c e", p=P))
        if has_b1:
            b1_sb = singles.tile([P, HC, E], F32)
            with nc.allow_non_contiguous_dma(reason="tiny one-time b1 load"):
                nc.sync.dma_start(b1_sb, b1_d[:].rearrange("e (c p) -> p c e", p=P))
        if has_b2:
            b2_sb = singles.tile([P, E, O], F32)
            nc.sync.dma_start(b2_sb, _bcast2(b2_d[:], P))

        # iotas
        iota_base_i = singles.tile([P, NT, E], I32)
        nc.gpsimd.iota(
            iota_base_i, pattern=[[CAPT, NT], [CAP, E]], base=0, channel_multiplier=0
        )
        iota_base = singles.tile([P, NT, E], F32)
        nc.vector.tensor_copy(iota_base, iota_base_i)
        iota_e_i = singles.tile([P, E], I32)
        nc.gpsimd.iota(iota_e_i, pattern=[[1, E]], base=0, channel_multiplier=0)
        iota_e = singles.tile([P, E], F32)
        nc.vector.tensor_copy(iota_e, iota_e_i)

        xT32 = singles.tile([P, DC, TC], F32)
        x16_all = singles.tile([P, NT, D], MM_DT)
        xTg_all = singles.tile([P, DC, E * CAP], MM_DT)
        slotk_all = singles.tile([P, NT, 2], I32)
        gates_all = singles.tile([P, NT, 2], F32)

        pr_all = psum_r.tile([P, NT, 2, E], F32)  # [:, tt, 0]: router, [:, tt, 1]: rank

        # ---- phase A: x load + transpose + batched router matmuls ----
        for tt in range(NT):
            xr = xload.tile([P, D], F32, tag="xr")
            nc.sync.dma_start(xr, x_d[:][tt * P:(tt + 1) * P, :])
            nc.vector.tensor_copy(x16_all[:, tt, :], xr)
            for dc in range(DC):
                pt = psum_t.tile([P, P], F32, tag="pt")
                nc.tensor.transpose(pt, xr[:, dc * P:(dc + 1) * P], ident)
                nc.vector.tensor_copy(xT32[:, dc, tt * P:(tt + 1) * P], pt)
            for dc in range(DC):
                nc.tensor.matmul(
                    pr_all[:, tt, 0, :],
                    lhsT=xT32[:, dc, tt * P:(tt + 1) * P], rhs=wg_sb[:, dc, :],
                    start=(dc == 0), stop=(dc == DC - 1),
                )

        # ---- weight prefetch (paced by pool rotation) ----
        w1_sbs, w2_sbs = [], []
        for e in range(E):
            w1_sb = w1pool.tile([P, DC, H], MM_DT, tag="w1")
            nc.sync.dma_start(w1_sb, w1_d[:][e].rearrange("(c p) h -> p c h", p=P))
            w1_sbs.append(w1_sb)
            w2_sb = w2pool.tile([P, HC, O], MM_DT, tag="w2")
            nc.sync.dma_start(w2_sb, w2_d[:][e].rearrange("(c p) o -> p c o", p=P))
            w2_sbs.append(w2_sb)

        # ---- phase B: per tile: top-2, rank, slots, dispatch scatter ----
        for tt in range(NT):
            ex = tmp.tile([P, E], F32, tag="ex")
            s = tmp.tile([P, 1], F32, tag="s")
            nc.scalar.activation(out=ex, in_=pr_all[:, tt, 0, :], func=AF.Exp, accum_out=s)
            top8 = tmp.tile([P, 8], F32, tag="top8")
            nc.vector.max(out=top8, in_=ex)
            mask = tmp.tile([P, E], F32, tag="mask")
            nc.vector.tensor_scalar(
                out=mask, in0=ex, scalar1=top8[:, 1:2], scalar2=None, op0=ALU.is_ge
            )
            # within-tile inclusive rank via triangular-ones matmul
            nc.tensor.matmul(
                pr_all[:, tt, 1, :], lhsT=tril, rhs=mask, start=True, stop=True
            )
            exclr = tmp.tile([P, E], F32, tag="exclr")
            nc.vector.tensor_sub(exclr, pr_all[:, tt, 1, :], mask)
            oh1 = tmp.tile([P, E], F32, tag="oh1")
            nc.vector.tensor_scalar(
                out=oh1, in0=ex, scalar1=top8[:, 0:1], scalar2=None, op0=ALU.is_equal
            )
            sel2 = tmp.tile([P, E], F32, tag="sel2")
            nc.vector.tensor_sub(sel2, mask, oh1)

            rk = tmp.tile([P, 2], F32, tag="rk")
            ek = tmp.tile([P, 2], F32, tag="ek")
            prod = tmp.tile([P, E], F32, tag="prod")
            nc.vector.tensor_mul(prod, oh1, exclr)
            nc.vector.reduce_sum(rk[:, 0:1], prod, axis=mybir.AxisListType.X)
            nc.vector.tensor_mul(prod, sel2, exclr)
            nc.vector.reduce_sum(rk[:, 1:2], prod, axis=mybir.AxisListType.X)
            nc.vector.tensor_mul(prod, oh1, iota_e)
            nc.vector.reduce_sum(ek[:, 0:1], prod, axis=mybir.AxisListType.X)
            nc.vector.tensor_mul(prod, sel2, iota_e)
            nc.vector.reduce_sum(ek[:, 1:2], prod, axis=mybir.AxisListType.X)

            # slot = e*CAP + tt*CAPT + rank
            slotf = tmp.tile([P, 2], F32, tag="slotf")
            nc.vector.tensor_scalar(
                out=slotf, in0=ek, scalar1=float(CAP), scalar2=float(tt * CAPT),
                op0=ALU.mult, op1=ALU.add,
            )
            nc.vector.tensor_add(slotf, slotf, rk)
            slotk_i = slotk_all[:, tt, :]
            nc.vector.tensor_copy(slotk_i, slotf)

            for k in range(2):
                nc.gpsimd.indirect_dma_start(
                    out=xg_d[:],
                    out_offset=IndirectOffsetOnAxis(ap=slotk_i[:, k:k + 1], axis=0),
                    in_=x16_all[:, tt, :],
                    in_offset=None,
                    bounds_check=E * CAP - 1,
                    oob_is_err=False,
                )

            rec = tmp.tile([P, 1], F32, tag="rec")
            nc.vector.reciprocal(rec, s)
            nc.vector.tensor_scalar_mul(gates_all[:, tt, :], top8[:, 0:2], rec)

        # ---- phase C: staging: load scattered rows + transpose (PE-light) ----
        for e in range(E):
            for sl in range(NS):
                xg = xload.tile([P, D], MM_DT, tag="xg")
                nc.sync.dma_start(
                    xg, xg_d[:][e * CAP + sl * P:e * CAP + (sl + 1) * P, :]
                )
                for dc in range(DC):
                    pt16 = psum_t.tile([P, P], MM_DT, tag="pt")
                    nc.tensor.transpose(pt16, xg[:, dc * P:(dc + 1) * P], ident16)
                    nc.vector.tensor_copy(
                        xTg_all[:, dc, e * CAP + sl * P:e * CAP + (sl + 1) * P],
                        pt16,
                    )

        # ---- phase D: per-expert MLP; y rows scattered by token id ----
        for e in range(E):
            w1_sb, w2_sb = w1_sbs[e], w2_sbs[e]
            h_sb = hpool.tile([P, HC, CAP], MM_DT, tag="h")
            for hc in range(HC):
                ph = psum_h.tile([P, CAP], F32)
                for dc in range(DC):
                    nc.tensor.matmul(
                        ph, lhsT=w1_sb[:, dc, hc * P:(hc + 1) * P],
                        rhs=xTg_all[:, dc, e * CAP:(e + 1) * CAP],
                        start=(dc == 0), stop=(dc == DC - 1),
                    )
                bias_ap = b1_sb[:, hc, e:e + 1] if has_b1 else 0.0
                nc.scalar.activation(
                    out=h_sb[:, hc, :], in_=ph, func=ACT_FN, bias=bias_ap
                )

            for sl in range(NS):
                py = psum_y.tile([P, O], F32)
                for hc in range(HC):
                    nc.tensor.matmul(
                        py, lhsT=h_sb[:, hc, sl * P:(sl + 1) * P], rhs=w2_sb[:, hc, :],
                        start=(hc == 0), stop=(hc == HC - 1),
                    )
                y16 = ypool.tile([P, O], MM_DT, tag="y16")
                if has_b2:
                    nc.vector.tensor_add(y16, py, b2_sb[:, e, :])
                else:
                    nc.vector.tensor_copy(y16, py)
                nc.sync.dma_start(
                    y_d[:][e * CAP + sl * P:e * CAP + (sl + 1) * P, :], y16
                )

        # ---- phase E: combine: gather both y rows by slot, gate, add, store ----
        for tt in range(NT):
            y1 = opool.tile([P, O], MM_DT, tag="y1")
            nc.gpsimd.indirect_dma_start(
                out=y1,
                out_offset=None,
                in_=y_d[:],
                in_offset=IndirectOffsetOnAxis(ap=slotk_all[:, tt, 0:1], axis=0),
                bounds_check=E * CAP - 1,
                oob_is_err=False,
            )
            y2 = opool.tile([P, O], MM_DT, tag="y2")
            nc.gpsimd.indirect_dma_start(
                out=y2,
                out_offset=None,
                in_=y_d[:],
                in_offset=IndirectOffsetOnAxis(ap=slotk_all[:, tt, 1:2], axis=0),
                bounds_check=E * CAP - 1,
                oob_is_err=False,
            )
            yo1 = opool.tile([P, O], F32, tag="yo1")
            yo2 = opool.tile([P, O], F32, tag="yo2")
            nc.vector.tensor_scalar_mul(yo1, y1, gates_all[:, tt, 0:1])
            nc.vector.tensor_scalar_mul(yo2, y2, gates_all[:, tt, 1:2])
            nc.vector.tensor_add(yo1, yo1, yo2)
            nc.sync.dma_start(out_d[:][tt * P:(tt + 1) * P, :], yo1)

    nc.finalize()
    return nc


_NC_CACHE: dict = {}


def _get_nc(has_b1: bool, has_b2: bool) -> bass.Bass:
    key = (has_b1, has_b2)
    if key not in _NC_CACHE:
        _NC_CACHE[key] = build_nc(has_b1, has_b2)
    return _NC_CACHE[key]


def kernel(x, Wg, W1, b1, W2, b2, _trace=False, _tmpdir=None):
    x = np.ascontiguousarray(np.asarray(x, dtype=np.float32))
    Wg = np.ascontiguousarray(np.asarray(Wg, dtype=np.float32))
    W1 = np.asarray(W1, dtype=np.float32)
    b1 = np.asarray(b1, dtype=np.float32)
    W2 = np.asarray(W2, dtype=np.float32)
    b2 = np.asarray(b2, dtype=np.float32)

    has_b1 = bool(np.any(b1))
    has_b2 = bool(np.any(b2))
    nc = _get_nc(has_b1, has_b2)

    xm = x.reshape(T, D)
    w1_bf = np.ascontiguousarray(W1.astype(NP_MM_DT))
    w2_bf = np.ascontiguousarray(W2.astype(NP_MM_DT))

    base = {"wg": Wg, "w1": w1_bf, "w2": w2_bf}
    if has_b1:
        base["b1"] = np.ascontiguousarray(b1)
    if has_b2:
        base["b2"] = np.ascontiguousarray(b2)

    in_maps = [
        {**base, "x": np.ascontiguousarray(xm[c * TC:(c + 1) * TC])}
        for c in range(N_CORES)
    ]
    res = run_bass_kernel_spmd(
        nc, in_maps, core_ids=list(range(N_CORES)), trace=_trace, tmpdir=_tmpdir
    )
    out = np.concatenate([res.results[c]["out"] for c in range(N_CORES)], axis=0)
    if _trace:
        kernel._last_result = res
    return out.reshape(B, S, O).astype(np.float32)


# revision 22
# speedup vs baseline: 1.2788x; 1.0450x over previous
"""MoE (top-2 of 8 experts) Trainium2 kernel, 8-core data-parallel over tokens.

Problem shapes (hardcoded): x [4, 2048, 512] f32, Wg [512, 8], W1 [8, 512, 1024],
b1 [8, 1024], W2 [8, 1024, 512], b2 [8, 512].  T = 8192 tokens, top-2 routing.

Strategy: shard tokens across the 8 cores (1024/core); replicate router and
expert weights (weights cast to bf16 host-side).  Indirect DMA on this part
drains through ~2 DMA engines (~45 GB/s), so the dispatch avoids it entirely:

  1. Per 128-token tile: load x, PE-transpose (f32 router path), router
     matmuls batched into one PSUM region per tile.
  2. Per tile: softmax/top-2 on DVE; within-tile rank via a triangular-ones
     matmul prefix sum; build a one-hot dispatch matrix P[tok, e*CAPT+rank]
     (exact bf16 0/1) plus the global slot ids for the combine.
  3. Dispatch on the PE: xTg[d, e*CAPT+rank] = x16_tt^T @ P_tt -- one matmul
     per (tile, d-chunk) gathers AND transposes every expert's rows at once;
     zero HBM round-trip, padded slots are exact zeros.
  4. Per expert: bf16 GEMM1 -> fused gelu_tanh(+b1) -> bf16 GEMM2 (+b2);
     y rows written slot-ordered in bf16, experts 0-3 to y_lo, 4-7 to y_hi.
  5. Combine per token tile: indirect-gather each token's two y rows -- the
     y_lo gathers issue right after expert 3 so they hide under experts 4-7's
     GEMMs; only the y_hi gathers trail the compute.  OOB-masked slot ids
     make each gather fetch only its half (dropped rows stay zero).
"""

from contextlib import ExitStack

import numpy as np
import ml_dtypes

import concourse.bass as bass
import concourse.tile as tile
from concourse import bacc, mybir
from concourse.bass import IndirectOffsetOnAxis
from concourse.bass_utils import run_bass_kernel_spmd
from concourse.masks import make_identity

P = 128
N_CORES = 8
B, S, D, H, O, E = 4, 2048, 512, 1024, 512, 8
T = B * S                    # 8192
TC = T // N_CORES            # 1024 tokens per core
DC = D // P                  # 4 D-chunks
HC = H // P                  # 8 H-chunks
NT = TC // P                 # 8 token tiles of 128
CAP = 384                    # per-expert token capacity (3 tiles of 128)
NS = CAP // P                # 3 slot tiles per expert
CAPT = CAP // NT             # 48: per-(tile, expert) local capacity
EH = E // 2                  # experts per y half
BIG = 1.0e6                  # OOB filler for masked slot ids

MM_DT = mybir.dt.bfloat16
NP_MM_DT = ml_dtypes.bfloat16
F32 = mybir.dt.float32
I32 = mybir.dt.int32
AF = mybir.ActivationFunctionType
ALU = mybir.AluOpType
ACT_FN = AF.Gelu_apprx_tanh  # simtest.py swaps this for Tanh (sim support)


def build_nc(has_b1: bool, has_b2: bool) -> bass.Bass:
    nc = bacc.Bacc()
    x_d = nc.declare_dram_parameter("x", [TC, D], F32, isOutput=False)
    wg_d = nc.declare_dram_parameter("wg", [D, E], F32, isOutput=False)
    w1_d = nc.declare_dram_parameter("w1", [E, D, H], MM_DT, isOutput=False)
    w2_d = nc.declare_dram_parameter("w2", [E, H, O], MM_DT, isOutput=False)
    if has_b1:
        b1_d = nc.declare_dram_parameter("b1", [E, H], F32, isOutput=False)
    if has_b2:
        b2_d = nc.declare_dram_parameter("b2", [E, O], F32, isOutput=False)
    out_d = nc.declare_dram_parameter("out", [TC, O], F32, isOutput=True)

    y_lo_d = nc.dram_tensor("ylo", [EH * CAP, O], MM_DT)
    y_hi_d = nc.dram_tensor("yhi", [EH * CAP, O], MM_DT)

    with ExitStack() as ctx:
        tc = ctx.enter_context(tile.TileContext(nc))
        singles = ctx.enter_context(tc.tile_pool(name="singles", bufs=1))
        xload = ctx.enter_context(tc.tile_pool(name="xload", bufs=4))
        w1pool = ctx.enter_context(tc.tile_pool(name="w1pool", bufs=4))
        w2pool = ctx.enter_context(tc.tile_pool(name="w2pool", bufs=4))
        hpool = ctx.enter_context(tc.tile_pool(name="hpool", bufs=2))
        tmp = ctx.enter_context(tc.tile_pool(name="tmp", bufs=6))
        ypool = ctx.enter_context(tc.tile_pool(name="ypool", bufs=4))
        lpool = ctx.enter_context(tc.tile_pool(name="lpool", bufs=NT))
        opool = ctx.enter_context(tc.tile_pool(name="opool", bufs=3))
        psum_t = ctx.enter_context(tc.tile_pool(name="psum_t", bufs=1, space="PSUM"))
        psum_r = ctx.enter_context(tc.tile_pool(name="psum_r", bufs=1, space="PSUM"))
        psum_g = ctx.enter_context(tc.tile_pool(name="psum_g", bufs=2, space="PSUM"))
        psum_h = ctx.enter_context(tc.tile_pool(name="psum_h", bufs=2, space="PSUM"))
        psum_y = ctx.enter_context(tc.tile_pool(name="psum_y", bufs=2, space="PSUM"))

        ident = singles.tile([P, P], F32)
        make_identity(nc, ident)

        # inclusive lower-triangular ones: tril[q, p] = 1.0 iff q <= p
        tril = singles.tile([P, P], F32)
        nc.gpsimd.memset(tril, 0.0)
        nc.gpsimd.affine_select(
            out=tril, in_=tril, compare_op=ALU.is_gt, fill=1.0,
            base=0, pattern=[[-1, P]], channel_multiplier=1,
        )

        wg_sb = singles.tile([P, DC, E], F32)
        nc.sync.dma_start(wg_sb, wg_d[:].rearrange("(c p) e -> p c e", p=P))
        if has_b1:
            b1_sb = singles.tile([P, HC, E], F32)
            with nc.allow_non_contiguous_dma(reason="tiny one-time b1 load"):
                nc.sync.dma_start(b1_sb, b1_d[:].rearrange("e (c p) -> p c e", p=P))
        if has_b2:
            b2_sb = singles.tile([P, E, O], F32)
            b2_ap = b2_d[:]
            b2_bcast = bass.AP(
                tensor=b2_ap.tensor, offset=b2_ap.offset, ap=[[0, P], *b2_ap.ap]
            )
            nc.sync.dma_start(b2_sb, b2_bcast)

        # iota48[p, j] = j (same every partition), for the one-hot rank compare
        iota48_i = singles.tile([P, CAPT], I32)
        nc.gpsimd.iota(iota48_i, pattern=[[1, CAPT]], base=0, channel_multiplier=0)
        iota48 = singles.tile([P, CAPT], F32)
        nc.vector.tensor_copy(iota48, iota48_i)
        iota_e_i = singles.tile([P, E], I32)
        nc.gpsimd.iota(iota_e_i, pattern=[[1, E]], base=0, channel_multiplier=0)
        iota_e = singles.tile([P, E], F32)
        nc.vector.tensor_copy(iota_e, iota_e_i)

        xT32 = singles.tile([P, DC, TC], F32)
        x16_all = singles.tile([P, NT, D], MM_DT)
        xTg_all = singles.tile([P, DC, E * CAP], MM_DT)
        p_all = singles.tile([P, NT, E * CAPT], MM_DT)
        slotlo_all = singles.tile([P, NT, 2], I32)
        slothi_all = singles.tile([P, NT, 2], I32)
        gates_all = singles.tile([P, NT, 2], F32)

        pr_all = psum_r.tile([P, NT, 2, E], F32)  # [:, tt, 0]: router, [:, tt, 1]: rank

        # ---- phase A: x load + transpose + batched router matmuls ----
        for tt in range(NT):
            xr = xload.tile([P, D], F32, tag="xr")
            nc.sync.dma_start(xr, x_d[:][tt * P:(tt + 1) * P, :])
            nc.vector.tensor_copy(x16_all[:, tt, :], xr)
            for dc in range(DC):
                pt = psum_t.tile([P, P], F32, tag="pt")
                nc.tensor.transpose(pt, xr[:, dc * P:(dc + 1) * P], ident)
                nc.vector.tensor_copy(xT32[:, dc, tt * P:(tt + 1) * P], pt)
            for dc in range(DC):
                nc.tensor.matmul(
                    pr_all[:, tt, 0, :],
                    lhsT=xT32[:, dc, tt * P:(tt + 1) * P], rhs=wg_sb[:, dc, :],
                    start=(dc == 0), stop=(dc == DC - 1),
                )

        # ---- weight prefetch (paced by pool rotation) ----
        w1_sbs, w2_sbs = [], []
        for e in range(E):
            w1_sb = w1pool.tile([P, DC, H], MM_DT, tag="w1")
            nc.sync.dma_start(w1_sb, w1_d[:][e].rearrange("(c p) h -> p c h", p=P))
            w1_sbs.append(w1_sb)
            w2_sb = w2pool.tile([P, HC, O], MM_DT, tag="w2")
            nc.sync.dma_start(w2_sb, w2_d[:][e].rearrange("(c p) o -> p c o", p=P))
            w2_sbs.append(w2_sb)

        # ---- phase B: per tile: top-2, rank, one-hot dispatch matrix ----
        for tt in range(NT):
            ex = tmp.tile([P, E], F32, tag="ex")
            s = tmp.tile([P, 1], F32, tag="s")
            nc.scalar.activation(
                out=ex, in_=pr_all[:, tt, 0, :], func=AF.Exp, accum_out=s
            )
            top8 = tmp.tile([P, 8], F32, tag="top8")
            nc.vector.max(out=top8, in_=ex)
            mask = tmp.tile([P, E], F32, tag="mask")
            nc.vector.tensor_scalar(
                out=mask, in0=ex, scalar1=top8[:, 1:2], scalar2=None, op0=ALU.is_ge
            )
            # within-tile inclusive rank via triangular-ones matmul
            nc.tensor.matmul(
                pr_all[:, tt, 1, :], lhsT=tril, rhs=mask, start=True, stop=True
            )
            # rank' = inclusive_rank * mask - 1: exclusive rank if selected, -1 if not
            rankp = tmp.tile([P, E], F32, tag="rankp")
            nc.vector.tensor_mul(rankp, pr_all[:, tt, 1, :], mask)
            nc.vector.tensor_scalar(
                out=rankp, in0=rankp, scalar1=1.0, scalar2=None, op0=ALU.subtract
            )
            # one-hot dispatch matrix: P[p, e*CAPT + r] = (r == rank'_e[p])
            for e in range(E):
                nc.vector.tensor_scalar(
                    out=p_all[:, tt, e * CAPT:(e + 1) * CAPT], in0=iota48,
                    scalar1=rankp[:, e:e + 1], scalar2=None, op0=ALU.is_equal,
                )

            # global slot ids (for the combine gathers): e*CAP + tt*CAPT + rank
            oh1 = tmp.tile([P, E], F32, tag="oh1")
            nc.vector.tensor_scalar(
                out=oh1, in0=ex, scalar1=top8[:, 0:1], scalar2=None, op0=ALU.is_equal
            )
            sel2 = tmp.tile([P, E], F32, tag="sel2")
            nc.vector.tensor_sub(sel2, mask, oh1)
            rk = tmp.tile([P, 2], F32, tag="rk")
            ek = tmp.tile([P, 2], F32, tag="ek")
            prod = tmp.tile([P, E], F32, tag="prod")
            nc.vector.tensor_mul(prod, oh1, rankp)
            nc.vector.reduce_sum(rk[:, 0:1], prod, axis=mybir.AxisListType.X)
            nc.vector.tensor_mul(prod, sel2, rankp)
            nc.vector.reduce_sum(rk[:, 1:2], prod, axis=mybir.AxisListType.X)
            nc.vector.tensor_mul(prod, oh1, iota_e)
            nc.vector.reduce_sum(ek[:, 0:1], prod, axis=mybir.AxisListType.X)
            nc.vector.tensor_mul(prod, sel2, iota_e)
            nc.vector.reduce_sum(ek[:, 1:2], prod, axis=mybir.AxisListType.X)
            slotf = tmp.tile([P, 2], F32, tag="slotf")
            nc.vector.tensor_scalar(
                out=slotf, in0=ek, scalar1=float(CAP), scalar2=float(tt * CAPT),
                op0=ALU.mult, op1=ALU.add,
            )
            nc.vector.tensor_add(slotf, slotf, rk)
            # masked per-half ids: lo = slot (or BIG), hi = slot - EH*CAP (or BIG)
            half = tmp.tile([P, 2], F32, tag="half")
            nc.vector.tensor_scalar(
                out=half, in0=slotf, scalar1=float(EH * CAP) - 0.5, scalar2=BIG,
                op0=ALU.is_ge, op1=ALU.mult,
            )
            lof = tmp.tile([P, 2], F32, tag="lof")
            nc.vector.tensor_add(lof, slotf, half)
            nc.vector.tensor_copy(slotlo_all[:, tt, :], lof)
            hif = tmp.tile([P, 2], F32, tag="hif")
            nc.vector.tensor_scalar(
                out=hif, in0=slotf, scalar1=float(EH * CAP) - 0.5, scalar2=BIG,
                op0=ALU.is_lt, op1=ALU.mult,
            )
            nc.vector.tensor_scalar(
                out=slotf, in0=slotf, scalar1=float(EH * CAP), scalar2=None,
                op0=ALU.subtract,
            )
            nc.vector.tensor_add(hif, hif, slotf)
            nc.vector.tensor_copy(slothi_all[:, tt, :], hif)

            rec = tmp.tile([P, 1], F32, tag="rec")
            nc.vector.reciprocal(rec, s)
            nc.vector.tensor_scalar_mul(gates_all[:, tt, :], top8[:, 0:2], rec)

        # ---- phase B2: PE dispatch: xTg[d, strips] = x16_tt^T @ P_tt ----
        for tt in range(NT):
            for dc in range(DC):
                pg = psum_g.tile([P, E * CAPT], F32, tag="pg")
                nc.tensor.matmul(
                    pg, lhsT=x16_all[:, tt, dc * P:(dc + 1) * P],
                    rhs=p_all[:, tt, :], start=True, stop=True,
                )
                base = xTg_all[:, dc, tt * CAPT:]
                dst = bass.AP(
                    tensor=base.tensor, offset=base.offset,
                    ap=[base.ap[0], [CAP, E], [1, CAPT]],
                )
                nc.vector.tensor_copy(dst, pg[:].rearrange("p (e c) -> p e c", e=E))

        # ---- phase C: per-expert MLP; y rows slot-ordered, split in halves ----
        for e in range(E):
            w1_sb, w2_sb = w1_sbs[e], w2_sbs[e]
            h_sb = hpool.tile([P, HC, CAP], MM_DT, tag="h")
            for hc in range(HC):
                ph = psum_h.tile([P, CAP], F32)
                for dc in range(DC):
                    nc.tensor.matmul(
                        ph, lhsT=w1_sb[:, dc, hc * P:(hc + 1) * P],
                        rhs=xTg_all[:, dc, e * CAP:(e + 1) * CAP],
                        start=(dc == 0), stop=(dc == DC - 1),
                    )
                bias_ap = b1_sb[:, hc, e:e + 1] if has_b1 else 0.0
                nc.scalar.activation(
                    out=h_sb[:, hc, :], in_=ph, func=ACT_FN, bias=bias_ap
                )

            y_dst = y_lo_d if e < EH else y_hi_d
            ebase = (e - (0 if e < EH else EH)) * CAP
            for sl in range(NS):
                py = psum_y.tile([P, O], F32)
                for hc in range(HC):
                    nc.tensor.matmul(
                        py, lhsT=h_sb[:, hc, sl * P:(sl + 1) * P], rhs=w2_sb[:, hc, :],
                        start=(hc == 0), stop=(hc == HC - 1),
                    )
                y16 = ypool.tile([P, O], MM_DT, tag="y16")
                if has_b2:
                    nc.vector.tensor_add(y16, py, b2_sb[:, e, :])
                else:
                    nc.vector.tensor_copy(y16, py)
                nc.sync.dma_start(
                    y_dst[:][ebase + sl * P:ebase + (sl + 1) * P, :], y16
                )

            # after the low half is written, start its combine gathers so they
            # overlap the high half's GEMMs
            if e == EH - 1:
                ylo_tiles = []
                for tt in range(NT):
                    pair = []
                    for k in range(2):
                        yl = lpool.tile([P, O], MM_DT, tag=f"ylo{k}")
                        nc.vector.memset(yl, 0.0)
                        nc.gpsimd.indirect_dma_start(
                            out=yl,
                            out_offset=None,
                            in_=y_lo_d[:],
                            in_offset=IndirectOffsetOnAxis(
                                ap=slotlo_all[:, tt, k:k + 1], axis=0
                            ),
                            bounds_check=EH * CAP - 1,
                            oob_is_err=False,
                        )
                        pair.append(yl)
                    ylo_tiles.append(pair)

        # ---- phase D: combine: high-half gathers, gate, add, store ----
        for tt in range(NT):
            yo = [None, None]
            for k in range(2):
                yh = opool.tile([P, O], MM_DT, tag=f"yhi{k}")
                nc.vector.memset(yh, 0.0)
                nc.gpsimd.indirect_dma_start(
                    out=yh,
                    out_offset=None,
                    in_=y_hi_d[:],
                    in_offset=IndirectOffsetOnAxis(
                        ap=slothi_all[:, tt, k:k + 1], axis=0
                    ),
                    bounds_check=EH * CAP - 1,
                    oob_is_err=False,
                )
                yk = opool.tile([P, O], F32, tag=f"yk{k}")
                nc.vector.tensor_add(yk, ylo_tiles[tt][k], yh)
                yo[k] = yk
            nc.vector.tensor_scalar_mul(yo[0], yo[0], gates_all[:, tt, 0:1])
            nc.vector.tensor_scalar_mul(yo[1], yo[1], gates_all[:, tt, 1:2])
            nc.vector.tensor_add(yo[0], yo[0], yo[1])
            nc.sync.dma_start(out_d[:][tt * P:(tt + 1) * P, :], yo[0])
            del yo

    nc.finalize()
    return nc


_NC_CACHE: dict = {}


def _get_nc(has_b1: bool, has_b2: bool) -> bass.Bass:
    key = (has_b1, has_b2)
    if key not in _NC_CACHE:
        _NC_CACHE[key] = build_nc(has_b1, has_b2)
    return _NC_CACHE[key]


def kernel(x, Wg, W1, b1, W2, b2, _trace=False, _tmpdir=None):
    x = np.ascontiguousarray(np.asarray(x, dtype=np.float32))
    Wg = np.ascontiguousarray(np.asarray(Wg, dtype=np.float32))
    W1 = np.asarray(W1, dtype=np.float32)
    b1 = np.asarray(b1, dtype=np.float32)
    W2 = np.asarray(W2, dtype=np.float32)
    b2 = np.asarray(b2, dtype=np.float32)

    has_b1 = bool(np.any(b1))
    has_b2 = bool(np.any(b2))
    nc = _get_nc(has_b1, has_b2)

    xm = x.reshape(T, D)
    w1_bf = np.ascontiguousarray(W1.astype(NP_MM_DT))
    w2_bf = np.ascontiguousarray(W2.astype(NP_MM_DT))

    base = {"wg": Wg, "w1": w1_bf, "w2": w2_bf}
    if has_b1:
        base["b1"] = np.ascontiguousarray(b1)
    if has_b2:
        base["b2"] = np.ascontiguousarray(b2)

    in_maps = [
        {**base, "x": np.ascontiguousarray(xm[c * TC:(c + 1) * TC])}
        for c in range(N_CORES)
    ]
    res = run_bass_kernel_spmd(
        nc, in_maps, core_ids=list(range(N_CORES)), trace=_trace, tmpdir=_tmpdir
    )
    out = np.concatenate([res.results[c]["out"] for c in range(N_CORES)], axis=0)
    if _trace:
        kernel._last_result = res
    return out.reshape(B, S, O).astype(np.float32)


# revision 29
# speedup vs baseline: 1.3322x; 1.0417x over previous
"""MoE (top-2 of 8 experts) Trainium2 kernel, 8-core data-parallel over tokens.

Problem shapes (hardcoded): x [4, 2048, 512] f32, Wg [512, 8], W1 [8, 512, 1024],
b1 [8, 1024], W2 [8, 1024, 512], b2 [8, 512].  T = 8192 tokens, top-2 routing.

Strategy: shard tokens across the 8 cores (1024/core); replicate router and
expert weights (weights cast to bf16 host-side).  Indirect DMA on this part
drains through ~2 DMA engines (~45 GB/s), so the dispatch avoids it entirely:

  1. Per 128-token tile: load x, PE-transpose (f32 router path), router
     matmuls batched into one PSUM region per tile.
  2. Per tile: softmax/top-2 on DVE; within-tile rank via a triangular-ones
     matmul prefix sum; build a one-hot dispatch matrix P[tok, e*CAPT+rank]
     (exact bf16 0/1) plus the global slot ids for the combine.
  3. Dispatch on the PE: xTg[d, e*CAPT+rank] = x16_tt^T @ P_tt -- one matmul
     per (tile, d-chunk) gathers AND transposes every expert's rows at once;
     zero HBM round-trip, padded slots are exact zeros.
  4. Per expert: bf16 GEMM1 -> fused gelu_tanh(+b1) -> bf16 GEMM2 (+b2);
     y rows written slot-ordered in bf16, experts 0-3 to y_lo, 4-7 to y_hi.
  5. Combine per token tile: indirect-gather each token's two y rows -- the
     y_lo gathers issue right after expert 3 so they hide under experts 4-7's
     GEMMs; only the y_hi gathers trail the compute.  OOB-masked slot ids
     make each gather fetch only its half (dropped rows stay zero).
"""

from contextlib import ExitStack

import numpy as np
import ml_dtypes

import concourse.bass as bass
import concourse.tile as tile
from concourse import bacc, mybir
from concourse.bass import IndirectOffsetOnAxis
from concourse.bass_utils import run_bass_kernel_spmd
from concourse.masks import make_identity

P = 128
N_CORES = 8
B, S, D, H, O, E = 4, 2048, 512, 1024, 512, 8
T = B * S                    # 8192
TC = T // N_CORES            # 1024 tokens per core
DC = D // P                  # 4 D-chunks
HC = H // P                  # 8 H-chunks
NT = TC // P                 # 8 token tiles of 128
CAP = 384                    # per-expert token capacity (3 tiles of 128)
NS = CAP // P                # 3 slot tiles per expert
CAPT = CAP // NT             # 48: per-(tile, expert) local capacity
EH = E // 2                  # experts per y half
BIG = 1.0e6                  # OOB filler for masked slot ids

MM_DT = mybir.dt.bfloat16
NP_MM_DT = ml_dtypes.bfloat16
F32 = mybir.dt.float32
I32 = mybir.dt.int32
AF = mybir.ActivationFunctionType
ALU = mybir.AluOpType
ACT_FN = AF.Gelu_apprx_tanh  # simtest.py swaps this for Tanh (sim support)


def build_nc(has_b1: bool, has_b2: bool) -> bass.Bass:
    nc = bacc.Bacc()
    x_d = nc.declare_dram_parameter("x", [TC, D], F32, isOutput=False)
    wg_d = nc.declare_dram_parameter("wg", [D, E], F32, isOutput=False)
    w1_d = nc.declare_dram_parameter("w1", [E, D, H], MM_DT, isOutput=False)
    w2_d = nc.declare_dram_parameter("w2", [E, H, O], MM_DT, isOutput=False)
    if has_b1:
        b1_d = nc.declare_dram_parameter("b1", [E, H], F32, isOutput=False)
    if has_b2:
        b2_d = nc.declare_dram_parameter("b2", [E, O], F32, isOutput=False)
    out_d = nc.declare_dram_parameter("out", [TC, O], F32, isOutput=True)

    y_lo_d = nc.dram_tensor("ylo", [EH * CAP, O], MM_DT)
    y_hi_d = nc.dram_tensor("yhi", [EH * CAP, O], MM_DT)

    with ExitStack() as ctx:
        tc = ctx.enter_context(tile.TileContext(nc))
        singles = ctx.enter_context(tc.tile_pool(name="singles", bufs=1))
        xload = ctx.enter_context(tc.tile_pool(name="xload", bufs=4))
        w1pool = ctx.enter_context(tc.tile_pool(name="w1pool", bufs=4))
        w2pool = ctx.enter_context(tc.tile_pool(name="w2pool", bufs=4))
        hpool = ctx.enter_context(tc.tile_pool(name="hpool", bufs=2))
        tmp = ctx.enter_context(tc.tile_pool(name="tmp", bufs=6))
        ypool = ctx.enter_context(tc.tile_pool(name="ypool", bufs=4))
        lpool = ctx.enter_context(tc.tile_pool(name="lpool", bufs=NT))
        hipool = ctx.enter_context(tc.tile_pool(name="hipool", bufs=NT))
        opool = ctx.enter_context(tc.tile_pool(name="opool", bufs=3))
        psum_t = ctx.enter_context(tc.tile_pool(name="psum_t", bufs=1, space="PSUM"))
        psum_r = ctx.enter_context(tc.tile_pool(name="psum_r", bufs=1, space="PSUM"))
        psum_g = ctx.enter_context(tc.tile_pool(name="psum_g", bufs=2, space="PSUM"))
        psum_h = ctx.enter_context(tc.tile_pool(name="psum_h", bufs=2, space="PSUM"))
        psum_y = ctx.enter_context(tc.tile_pool(name="psum_y", bufs=2, space="PSUM"))

        ident = singles.tile([P, P], F32)
        make_identity(nc, ident)

        # inclusive lower-triangular ones: tril[q, p] = 1.0 iff q <= p
        tril = singles.tile([P, P], F32)
        nc.gpsimd.memset(tril, 0.0)
        nc.gpsimd.affine_select(
            out=tril, in_=tril, compare_op=ALU.is_gt, fill=1.0,
            base=0, pattern=[[-1, P]], channel_multiplier=1,
        )

        wg_sb = singles.tile([P, DC, E], F32)
        nc.sync.dma_start(wg_sb, wg_d[:].rearrange("(c p) e -> p c e", p=P))
        if has_b1:
            b1_sb = singles.tile([P, HC, E], F32)
            with nc.allow_non_contiguous_dma(reason="tiny one-time b1 load"):
                nc.sync.dma_start(b1_sb, b1_d[:].rearrange("e (c p) -> p c e", p=P))
        if has_b2:
            b2_sb = singles.tile([P, E, O], F32)
            b2_ap = b2_d[:]
            b2_bcast = bass.AP(
                tensor=b2_ap.tensor, offset=b2_ap.offset, ap=[[0, P], *b2_ap.ap]
            )
            nc.sync.dma_start(b2_sb, b2_bcast)

        # iota384[p, j] = j (same every partition), for the one-hot col compare
        iota384_i = singles.tile([P, E * CAPT], I32)
        nc.gpsimd.iota(iota384_i, pattern=[[1, E * CAPT]], base=0, channel_multiplier=0)
        iota384 = singles.tile([P, E * CAPT], F32)
        nc.vector.tensor_copy(iota384, iota384_i)
        iota_e_i = singles.tile([P, E], I32)
        nc.gpsimd.iota(iota_e_i, pattern=[[1, E]], base=0, channel_multiplier=0)
        iota_e = singles.tile([P, E], F32)
        nc.vector.tensor_copy(iota_e, iota_e_i)

        xT32 = singles.tile([P, DC, TC], F32)
        x16_all = singles.tile([P, NT, D], MM_DT)
        xTg_all = singles.tile([P, DC, E * CAP], MM_DT)
        p_all = singles.tile([P, NT, E * CAPT], MM_DT)
        slotlo_all = singles.tile([P, NT, 2], I32)
        slothi_all = singles.tile([P, NT, 2], I32)
        gates_all = singles.tile([P, NT, 2], F32)

        pr_all = psum_r.tile([P, NT, 2, E], F32)  # [:, tt, 0]: router, [:, tt, 1]: rank

        # ---- phase A: x load + transpose + batched router matmuls ----
        for tt in range(NT):
            xr = xload.tile([P, D], F32, tag="xr")
            nc.sync.dma_start(xr, x_d[:][tt * P:(tt + 1) * P, :])
            nc.vector.tensor_copy(x16_all[:, tt, :], xr)
            for dc in range(DC):
                pt = psum_t.tile([P, P], F32, tag="pt")
                nc.tensor.transpose(pt, xr[:, dc * P:(dc + 1) * P], ident)
                nc.vector.tensor_copy(xT32[:, dc, tt * P:(tt + 1) * P], pt)
            for dc in range(DC):
                nc.tensor.matmul(
                    pr_all[:, tt, 0, :],
                    lhsT=xT32[:, dc, tt * P:(tt + 1) * P], rhs=wg_sb[:, dc, :],
                    start=(dc == 0), stop=(dc == DC - 1),
                )

        # ---- weight prefetch (paced by pool rotation) ----
        w1_sbs, w2_sbs = [], []
        for e in range(E):
            w1_sb = w1pool.tile([P, DC, H], MM_DT, tag="w1")
            nc.sync.dma_start(w1_sb, w1_d[:][e].rearrange("(c p) h -> p c h", p=P))
            w1_sbs.append(w1_sb)
            w2_sb = w2pool.tile([P, HC, O], MM_DT, tag="w2")
            nc.sync.dma_start(w2_sb, w2_d[:][e].rearrange("(c p) o -> p c o", p=P))
            w2_sbs.append(w2_sb)

        # ---- phase B: per tile: top-2, rank, one-hot dispatch matrix ----
        for tt in range(NT):
            ex = tmp.tile([P, E], F32, tag="ex")
            s = tmp.tile([P, 1], F32, tag="s")
            nc.scalar.activation(
                out=ex, in_=pr_all[:, tt, 0, :], func=AF.Exp, accum_out=s
            )
            top8 = tmp.tile([P, 8], F32, tag="top8")
            nc.vector.max(out=top8, in_=ex)
            mask = tmp.tile([P, E], F32, tag="mask")
            nc.vector.tensor_scalar(
                out=mask, in0=ex, scalar1=top8[:, 1:2], scalar2=None, op0=ALU.is_ge
            )
            # within-tile inclusive rank via triangular-ones matmul
            nc.tensor.matmul(
                pr_all[:, tt, 1, :], lhsT=tril, rhs=mask, start=True, stop=True
            )
            # rank' = inclusive_rank * mask - 1: exclusive rank if selected, -1 if not
            rankp = tmp.tile([P, E], F32, tag="rankp")
            nc.vector.tensor_mul(rankp, pr_all[:, tt, 1, :], mask)
            nc.vector.tensor_scalar(
                out=rankp, in0=rankp, scalar1=1.0, scalar2=None, op0=ALU.subtract
            )
            oh1 = tmp.tile([P, E], F32, tag="oh1")
            nc.vector.tensor_scalar(
                out=oh1, in0=ex, scalar1=top8[:, 0:1], scalar2=None, op0=ALU.is_equal
            )
            sel2 = tmp.tile([P, E], F32, tag="sel2")
            nc.vector.tensor_sub(sel2, mask, oh1)
            rk = tmp.tile([P, 2], F32, tag="rk")
            ek = tmp.tile([P, 2], F32, tag="ek")
            prod = tmp.tile([P, E], F32, tag="prod")
            nc.vector.tensor_mul(prod, oh1, rankp)
            nc.vector.reduce_sum(rk[:, 0:1], prod, axis=mybir.AxisListType.X)
            nc.vector.tensor_mul(prod, sel2, rankp)
            nc.vector.reduce_sum(rk[:, 1:2], prod, axis=mybir.AxisListType.X)
            nc.vector.tensor_mul(prod, oh1, iota_e)
            nc.vector.reduce_sum(ek[:, 0:1], prod, axis=mybir.AxisListType.X)
            nc.vector.tensor_mul(prod, sel2, iota_e)
            nc.vector.reduce_sum(ek[:, 1:2], prod, axis=mybir.AxisListType.X)
            # one-hot dispatch matrix: P[p, e*CAPT + r] = 1 at both top-k cols
            cp = tmp.tile([P, 2], F32, tag="cp")
            nc.vector.tensor_scalar(
                out=cp, in0=ek, scalar1=float(CAPT), scalar2=None, op0=ALU.mult
            )
            nc.vector.tensor_add(cp, cp, rk)
            ptmp = tmp.tile([P, E * CAPT], MM_DT, tag="ptmp")
            nc.vector.tensor_scalar(
                out=p_all[:, tt, :], in0=iota384, scalar1=cp[:, 0:1], scalar2=None,
                op0=ALU.is_equal,
            )
            nc.vector.tensor_scalar(
                out=ptmp, in0=iota384, scalar1=cp[:, 1:2], scalar2=None,
                op0=ALU.is_equal,
            )
            nc.vector.tensor_add(p_all[:, tt, :], p_all[:, tt, :], ptmp)
            slotf = tmp.tile([P, 2], F32, tag="slotf")
            nc.vector.tensor_scalar(
                out=slotf, in0=ek, scalar1=float(CAP), scalar2=float(tt * CAPT),
                op0=ALU.mult, op1=ALU.add,
            )
            nc.vector.tensor_add(slotf, slotf, rk)
            # masked per-half ids: lo = slot (or BIG), hi = slot - EH*CAP (or BIG)
            half = tmp.tile([P, 2], F32, tag="half")
            nc.vector.tensor_scalar(
                out=half, in0=slotf, scalar1=float(EH * CAP) - 0.5, scalar2=BIG,
                op0=ALU.is_ge, op1=ALU.mult,
            )
            lof = tmp.tile([P, 2], F32, tag="lof")
            nc.vector.tensor_add(lof, slotf, half)
            nc.vector.tensor_copy(slotlo_all[:, tt, :], lof)
            hif = tmp.tile([P, 2], F32, tag="hif")
            nc.vector.tensor_scalar(
                out=hif, in0=slotf, scalar1=float(EH * CAP) - 0.5, scalar2=BIG,
                op0=ALU.is_lt, op1=ALU.mult,
            )
            nc.vector.tensor_scalar(
                out=slotf, in0=slotf, scalar1=float(EH * CAP), scalar2=None,
                op0=ALU.subtract,
            )
            nc.vector.tensor_add(hif, hif, slotf)
            nc.vector.tensor_copy(slothi_all[:, tt, :], hif)

            rec = tmp.tile([P, 1], F32, tag="rec")
            nc.vector.reciprocal(rec, s)
            nc.vector.tensor_scalar_mul(gates_all[:, tt, :], top8[:, 0:2], rec)

        # ---- phase B2: PE dispatch: xTg[d, strips] = x16_tt^T @ P_tt ----
        for tt in range(NT):
            for dc in range(DC):
                pg = psum_g.tile([P, E * CAPT], F32, tag="pg")
                nc.tensor.matmul(
                    pg, lhsT=x16_all[:, tt, dc * P:(dc + 1) * P],
                    rhs=p_all[:, tt, :], start=True, stop=True,
                )
                base = xTg_all[:, dc, tt * CAPT:]
                dst = bass.AP(
                    tensor=base.tensor, offset=base.offset,
                    ap=[base.ap[0], [CAP, E], [1, CAPT]],
                )
                # scalar-engine copy keeps the DVE free for the router chain
                nc.scalar.activation(
                    out=dst, in_=pg[:].rearrange("p (e c) -> p e c", e=E), func=AF.Copy
                )

        # ---- phase C: per-expert MLP; y rows slot-ordered, split in halves ----
        for e in range(E):
            w1_sb, w2_sb = w1_sbs[e], w2_sbs[e]
            h_sb = hpool.tile([P, HC, CAP], MM_DT, tag="h")
            for hc in range(HC):
                ph = psum_h.tile([P, CAP], F32)
                for dc in range(DC):
                    nc.tensor.matmul(
                        ph, lhsT=w1_sb[:, dc, hc * P:(hc + 1) * P],
                        rhs=xTg_all[:, dc, e * CAP:(e + 1) * CAP],
                        start=(dc == 0), stop=(dc == DC - 1),
                    )
                bias_ap = b1_sb[:, hc, e:e + 1] if has_b1 else 0.0
                nc.scalar.activation(
                    out=h_sb[:, hc, :], in_=ph, func=ACT_FN, bias=bias_ap
                )

            y_dst = y_lo_d if e < EH else y_hi_d
            ebase = (e - (0 if e < EH else EH)) * CAP
            for sl in range(NS):
                py = psum_y.tile([P, O], F32)
                for hc in range(HC):
                    nc.tensor.matmul(
                        py, lhsT=h_sb[:, hc, sl * P:(sl + 1) * P], rhs=w2_sb[:, hc, :],
                        start=(hc == 0), stop=(hc == HC - 1),
                    )
                y16 = ypool.tile([P, O], MM_DT, tag="y16")
                if has_b2:
                    nc.vector.tensor_add(y16, py, b2_sb[:, e, :])
                else:
                    nc.vector.tensor_copy(y16, py)
                nc.sync.dma_start(
                    y_dst[:][ebase + sl * P:ebase + (sl + 1) * P, :], y16
                )

            # after the low half is written, start its combine gathers so they
            # overlap the high half's GEMMs
            if e == EH - 1:
                ylo_tiles = []
                for tt in range(NT):
                    pair = []
                    for k in range(2):
                        yl = lpool.tile([P, O], MM_DT, tag=f"ylo{k}")
                        nc.gpsimd.memset(yl, 0.0)
                        nc.gpsimd.indirect_dma_start(
                            out=yl,
                            out_offset=None,
                            in_=y_lo_d[:],
                            in_offset=IndirectOffsetOnAxis(
                                ap=slotlo_all[:, tt, k:k + 1], axis=0
                            ),
                            bounds_check=EH * CAP - 1,
                            oob_is_err=False,
                        )
                        pair.append(yl)
                    ylo_tiles.append(pair)
                # pre-zero the high-half gather tiles while gpsimd is hidden
                # under the remaining experts' GEMMs
                yhi_tiles = []
                for tt in range(NT):
                    pair = []
                    for k in range(2):
                        yh = hipool.tile([P, O], MM_DT, tag=f"yhi{k}")
                        nc.gpsimd.memset(yh, 0.0)
                        pair.append(yh)
                    yhi_tiles.append(pair)

        # ---- phase D: combine: high-half gathers, gate, add, store ----
        for tt in range(NT):
            yo = [None, None]
            for k in range(2):
                yh = yhi_tiles[tt][k]
                nc.gpsimd.indirect_dma_start(
                    out=yh,
                    out_offset=None,
                    in_=y_hi_d[:],
                    in_offset=IndirectOffsetOnAxis(
                        ap=slothi_all[:, tt, k:k + 1], axis=0
                    ),
                    bounds_check=EH * CAP - 1,
                    oob_is_err=False,
                )
                yk = opool.tile([P, O], F32, tag=f"yk{k}")
                nc.vector.tensor_add(yk, ylo_tiles[tt][k], yh)
                # gate on the scalar engine (idle at the tail)
                nc.scalar.activation(
                    out=yk, in_=yk, func=AF.Copy, scale=gates_all[:, tt, k:k + 1]
                )
                yo[k] = yk
            nc.vector.tensor_add(yo[0], yo[0], yo[1])
            nc.sync.dma_start(out_d[:][tt * P:(tt + 1) * P, :], yo[0])
            del yo

    nc.finalize()
    return nc


_NC_CACHE: dict = {}


def _get_nc(has_b1: bool, has_b2: bool) -> bass.Bass:
    key = (has_b1, has_b2)
    if key not in _NC_CACHE:
        _NC_CACHE[key] = build_nc(has_b1, has_b2)
    return _NC_CACHE[key]


def kernel(x, Wg, W1, b1, W2, b2, _trace=False, _tmpdir=None):
    x = np.ascontiguousarray(np.asarray(x, dtype=np.float32))
    Wg = np.ascontiguousarray(np.asarray(Wg, dtype=np.float32))
    W1 = np.asarray(W1, dtype=np.float32)
    b1 = np.asarray(b1, dtype=np.float32)
    W2 = np.asarray(W2, dtype=np.float32)
    b2 = np.asarray(b2, dtype=np.float32)

    has_b1 = bool(np.any(b1))
    has_b2 = bool(np.any(b2))
    nc = _get_nc(has_b1, has_b2)

    xm = x.reshape(T, D)
    w1_bf = np.ascontiguousarray(W1.astype(NP_MM_DT))
    w2_bf = np.ascontiguousarray(W2.astype(NP_MM_DT))

    base = {"wg": Wg, "w1": w1_bf, "w2": w2_bf}
    if has_b1:
        base["b1"] = np.ascontiguousarray(b1)
    if has_b2:
        base["b2"] = np.ascontiguousarray(b2)

    in_maps = [
        {**base, "x": np.ascontiguousarray(xm[c * TC:(c + 1) * TC])}
        for c in range(N_CORES)
    ]
    res = run_bass_kernel_spmd(
        nc, in_maps, core_ids=list(range(N_CORES)), trace=_trace, tmpdir=_tmpdir
    )
    out = np.concatenate([res.results[c]["out"] for c in range(N_CORES)], axis=0)
    if _trace:
        kernel._last_result = res
    return out.reshape(B, S, O).astype(np.float32)
